# revision 26
# baseline (speedup 1.0000x reference)
"""ChannelAttention Trainium2 kernel (Bass/Tile), data-parallel over batch.

Problem shapes (hardcoded):
  x      [8, 4096, 768] fp32
  w_qkv  [2304, 768]    fp32
  w_proj [768, 768]     fp32
  b_proj [768]          fp32
  out    [8, 4096, 768] fp32

Reference (per batch b, 8 groups of 96 channels):
  qkv = x @ w_qkv.T ; q *= N**-0.5
  attn_g = softmax(q_g.T @ k_g, axis=-1)     # [96, 96], contracts over N
  out_g  = attn_g @ v_g.T                    # [96, N]
  y = out @ w_proj.T + b_proj
Sharding: batch b -> core b (8 cores SPMD, no collectives).

Algebraic restructure: channel attention collapses around two small
matrices --
  G = X^T X                      [768, 768]   (Gram, symmetric)
  attn_g = softmax(Wq_s G Wk^T)  (per group, [96, 96])
  M = Wv^T BD(attn)^T WprojT     [768, 768]
  y = x @ M + b_proj
so the per-token work is one 768-contraction pass for G (x natural
layout) and one for y (x^T layout), plus O(768^3)-ish small matmuls
once per core.

v5 over v4:
  - x^T uploaded from host (layout prep), killing 192 PE transposes and
    their PSUM->SBUF drain copies.
  - Gram accumulated in 8 persistent PSUM groups across all 32 token
    tiles (PSUM holds the full upper triangle row-chunks): no SBUF
    fp32 accumulation traffic at all.
  - PE clock-ramp warmup: dummy matmuls keep the tensor engine busy
    through DMA/runtime init so real work runs at max p-state.
  - x loads on the sync queue (scalar queue starts with a 1.3us
    activation-table load that used to delay them).
  - y stored fp16 (host casts back to fp32), stores spread across
    three DMA queues.
All matmul operands fp16 (full PE rate), fp32 accumulation in PSUM;
softmax in fp32.  Host pre-work: fp16 casts, fold N**-0.5 into Wq,
transposes of x / q,k weight halves / w_proj (layout prep only).
"""

import numpy as np

B, N, C = 8, 4096, 768
G = 8
GC = C // G          # 96
NCORES = 8
NT = N // 128        # 32 token tiles
CC = C // 128        # 6 chunks of the channel dim
QSCALE = float(N) ** -0.5  # 1/64
NWARM = 20           # PE clock-ramp warmup matmuls

_CACHE = {}

# Gram upper-triangle slices per row-chunk a: (a, off, w, bank, boff) with
# w<=512, covering cols [a*128, 768) for each a.  The 8 accumulation
# regions are packed into 6 PSUM banks (512 fp32 cols each), leaving room
# for a persistent warmup bank + later pools.
GRAM_GROUPS = [
    (0, 0, 512, 0, 0),
    (0, 512, 256, 4, 0),
    (1, 128, 512, 1, 0),
    (1, 640, 128, 3, 384),
    (2, 256, 512, 2, 0),
    (3, 384, 384, 3, 0),
    (4, 512, 256, 4, 256),
    (5, 640, 128, 5, 0),
]


def _build_nc():
    import concourse.bass as bass
    import concourse.mybir as mybir
    import concourse.tile as tile
    from concourse import bacc

    fp16 = mybir.dt.float16
    fp32 = mybir.dt.float32

    nc = bacc.Bacc(
        "TRN2", target_bir_lowering=False, debug=False, num_devices=NCORES
    )

    xh = nc.dram_tensor("xh", [N, C], fp16, kind="ExternalInput").ap()
    xTh = nc.dram_tensor("xTh", [C, N], fp16, kind="ExternalInput").ap()
    # q/k halves of w_qkv, transposed to [c, 2*768], q pre-scaled
    wqkT = nc.dram_tensor("wqkT", [C, 2 * C], fp16, kind="ExternalInput").ap()
    # v rows of w_qkv in natural [d, a] layout
    wv = nc.dram_tensor("wv", [C, C], fp16, kind="ExternalInput").ap()
    wprojT = nc.dram_tensor("wprojT", [C, C], fp16, kind="ExternalInput").ap()
    bproj = nc.dram_tensor("bproj", [C], fp32, kind="ExternalInput").ap()
    id16d = nc.dram_tensor("id16", [128, 128], fp16, kind="ExternalInput").ap()
    y = nc.dram_tensor("y", [N, C], fp16, kind="ExternalOutput").ap()

    groups = GRAM_GROUPS

    with tile.TileContext(nc) as tc:
        from contextlib import ExitStack

        with ExitStack() as ctx:
            weights = ctx.enter_context(tc.tile_pool(name="weights", bufs=1))
            persist = ctx.enter_context(tc.tile_pool(name="persist", bufs=1))
            xn_pool = ctx.enter_context(tc.tile_pool(name="xn", bufs=24))
            ysb_pool = ctx.enter_context(tc.tile_pool(name="ysb", bufs=6))
            sm_pool = ctx.enter_context(tc.tile_pool(name="sm", bufs=4))

            # ---- static weight tiles ----
            warm = weights.tile([128, 512], fp16, name="warm")
            ident16 = weights.tile([128, 128], fp16, name="ident16")
            wqk_sb = [
                weights.tile([128, 2 * C], fp16, name=f"wqk_{a}")
                for a in range(CC)
            ]
            wv_sb = [
                weights.tile([128, C], fp16, name=f"wv_{dd}") for dd in range(CC)
            ]
            wpg_sb = [
                weights.tile([GC, C], fp16, name=f"wpg_{g}") for g in range(G)
            ]
            bias_sb = weights.tile([128, C], fp32, name="bias_sb")

            # ---- persistent intermediates ----
            G16 = [
                persist.tile([128, C], fp16, name=f"G16_{a}") for a in range(CC)
            ]
            xT6 = [
                persist.tile([128, N], fp16, name=f"xT_{a}") for a in range(CC)
            ]
            e16 = [
                persist.tile([GC, GC], fp16, name=f"e16_{g}") for g in range(G)
            ]
            M1_sb = [
                persist.tile([128, C], fp16, name=f"m1_{a}") for a in range(CC)
            ]
            P6 = [persist.tile([128, C], fp16, name=f"P_{dd}") for dd in range(CC)]
            M_sb = [
                persist.tile([128, C], fp16, name=f"M_{a}") for a in range(CC)
            ]

            # warm tile zeroed by vector at t~0
            nc.vector.memset(warm, 0.0)
            # no-dep scalar op at t~0: hoists the scalar engine's 1.3us
            # ACT_TABLE_LOAD into the runtime-init head (otherwise it runs
            # right before the first real scalar copy, mid-kernel).
            scal_scratch = weights.tile([128, 8], fp16, name="scal_scratch")
            nc.scalar.copy(out=scal_scratch, in_=warm[:, :8])

            # persistent warmup PSUM bank: dummy matmuls with no data deps,
            # used to keep the PE busy (and at max p-state) across phase
            # transitions.
            pswarm = ctx.enter_context(
                tc.tile_pool(name="pswarm", bufs=1, space="PSUM")
            )
            warm_ps = pswarm.tile([128, 128], fp32, name="warm_ps")

            def pe_fill(n, tag):
                for i in range(n):
                    nc.tensor.matmul(
                        warm_ps,
                        warm[:, :128],
                        warm[:, :128],
                        start=True,
                        stop=True,
                    )

            # ---- DMA program: everything on the sync ring in priority
            # order (in-order ring => x never starves behind bulk loads).
            # x tiles are emitted in the gram loop below; then xT, wqk,
            # wv, wpg follow.  gpsimd ring: ident + bias (tiny).
            nc.gpsimd.dma_start(out=ident16, in_=id16d)
            bias_bcast = bass.AP(
                tensor=bproj.tensor,
                offset=bproj.offset,
                ap=[[0, 128]] + [list(p) for p in bproj.ap],
            )
            nc.gpsimd.dma_start(out=bias_sb, in_=bias_bcast)

            # ---- phase 1: Gram in 8 persistent PSUM regions (6 banks) ----
            with tc.tile_pool(name="psg", bufs=1, space="PSUM") as psg:
                gbank = [
                    psg.tile([128, 512], fp32, name=f"gbank_{b}")
                    for b in range(6)
                ]
                gacc = [
                    gbank[bank][:, boff : boff + w]
                    for (a, off, w, bank, boff) in groups
                ]

                # PE warmup until the first x tile lands (~9.8us)
                pe_fill(NWARM, "warmup")

                # per PSUM bank there is ONE accumulation group (start
                # zeroes the whole bank): start on the bank's first-touch
                # matmul of ti=0, stop on its last touch of ti=NT-1.
                bank_first = {}
                bank_last = {}
                for gi, (a, off, w, bank, boff) in enumerate(groups):
                    bank_first.setdefault(bank, gi)
                    bank_last[bank] = gi
                for ti in range(NT):
                    xtile = xn_pool.tile([128, C], fp16, tag="xn", name=f"xn_{ti}")
                    nc.sync.dma_start(
                        out=xtile, in_=xh[ti * 128 : (ti + 1) * 128, :]
                    )
                    for gi, (a, off, w, bank, boff) in enumerate(groups):
                        nc.tensor.matmul(
                            gacc[gi],
                            xtile[:, a * 128 : (a + 1) * 128],
                            xtile[:, off : off + w],
                            start=(ti == 0 and bank_first[bank] == gi),
                            stop=(ti == NT - 1 and bank_last[bank] == gi),
                            skip_group_check=True,
                        )

                # bulk loads queued on the sync ring behind all of x
                for a in range(CC):
                    nc.sync.dma_start(
                        out=xT6[a], in_=xTh[a * 128 : (a + 1) * 128, :]
                    )
                for a in range(CC):
                    nc.sync.dma_start(
                        out=wqk_sb[a], in_=wqkT[a * 128 : (a + 1) * 128, :]
                    )
                for dd in range(CC):
                    nc.sync.dma_start(
                        out=wv_sb[dd], in_=wv[dd * 128 : (dd + 1) * 128, :]
                    )
                for g in range(G):
                    nc.sync.dma_start(
                        out=wpg_sb[g], in_=wprojT[g * GC : (g + 1) * GC, :]
                    )

                # drain casts to fp16: each group split in two halves,
                # spread over vector/scalar/gpsimd so no engine serializes.
                eng = 0
                dr_engines = [
                    lambda o, i: nc.vector.tensor_copy(o, i),
                    lambda o, i: nc.scalar.copy(out=o, in_=i),
                ]
                for gi, (a, off, w, bank, boff) in enumerate(groups):
                    h = w // 2
                    for (lo, hi) in ((0, h), (h, w)):
                        dr_engines[eng % 2](
                            G16[a][:, off + lo : off + hi], gacc[gi][:, lo:hi]
                        )
                        eng += 1

                # keep the PE busy while the drains land (sem latency)
                pe_fill(32, "drain_fill")

            with tc.tile_pool(name="psb", bufs=5, space="PSUM") as psb:
                # ---- phase 2a: mirror lower G16 blocks (G symmetric) ----
                for a in range(CC):
                    for b_ in range(a + 1, CC):
                        tp = psb.tile(
                            [128, 128], fp16, tag="big", name=f"mir_{a}_{b_}"
                        )
                        nc.tensor.transpose(
                            tp, G16[a][:, b_ * 128 : (b_ + 1) * 128], ident16
                        )
                        if (a + b_) % 2 == 0:
                            nc.vector.tensor_copy(
                                G16[b_][:, a * 128 : (a + 1) * 128], tp
                            )
                        else:
                            nc.scalar.copy(
                                out=G16[b_][:, a * 128 : (a + 1) * 128], in_=tp
                            )

                # ---- phase 2b: M1 = G Wk^T (half-major), then per-group
                # A_g = Wq_s_g^T M1_g + softmax ----
                for half in range(2):
                    hsl = slice(half * 384, (half + 1) * 384)
                    for a in range(CC):
                        m1_ps = psb.tile(
                            [128, 384], fp32, tag="big", name=f"m1ps_{a}_{half}"
                        )
                        for b_ in range(CC):
                            nc.tensor.matmul(
                                m1_ps,
                                G16[b_][:, a * 128 : (a + 1) * 128],
                                wqk_sb[b_][
                                    :, 768 + half * 384 : 768 + (half + 1) * 384
                                ],
                                start=(b_ == 0),
                                stop=(b_ == CC - 1),
                            )
                        if (a + half) % 2 == 0:
                            nc.scalar.copy(out=M1_sb[a][:, hsl], in_=m1_ps)
                        else:
                            nc.vector.tensor_copy(M1_sb[a][:, hsl], m1_ps)

                for g in range(G):
                    a_ps = psb.tile([GC, GC], fp32, tag="big", name=f"aps_{g}")
                    for a in range(CC):
                        nc.tensor.matmul(
                            a_ps,
                            wqk_sb[a][:, g * GC : (g + 1) * GC],
                            M1_sb[a][:, g * GC : (g + 1) * GC],
                            start=(a == 0),
                            stop=(a == CC - 1),
                        )

                    nm = sm_pool.tile([GC, 1], fp32, tag="nm", name=f"nm_{g}")
                    nc.vector.tensor_reduce(
                        out=nm,
                        in_=a_ps,
                        axis=mybir.AxisListType.X,
                        op=mybir.AluOpType.max,
                        negate=True,
                    )
                    e_t = sm_pool.tile([GC, GC], fp32, tag="e", name=f"e_{g}")
                    ssum = sm_pool.tile([GC, 1], fp32, tag="ssum", name=f"ssum_{g}")
                    nc.scalar.activation(
                        e_t,
                        a_ps,
                        mybir.ActivationFunctionType.Exp,
                        bias=nm,
                        scale=1.0,
                        accum_out=ssum,
                    )
                    rs = sm_pool.tile([GC, 1], fp32, tag="rs", name=f"rs_{g}")
                    nc.vector.reciprocal(rs, ssum)
                    nc.vector.tensor_scalar_mul(e16[g], e_t, rs)

                # ---- phase 2c: P = BD(attn)^T WprojT in 128-aligned
                # d-chunks (piece matmuls land at their global-d psum
                # partitions via tile_position); M = Wv^T P with K=128 ----
                def d_pieces(dd):
                    raw = []
                    for g in range(G):
                        lo, hi = g * GC, (g + 1) * GC
                        r0 = max(0, 128 * dd - lo)
                        r1 = min(GC, 128 * (dd + 1) - lo)
                        if r0 < r1:
                            raw.append((g, r0, r1, lo + r0 - 128 * dd))
                    # split pieces that violate PE col-group placement rules
                    # (M<=32 at {0,32,64,96}; M<=64 at {0,64}; M>64 only at 0)
                    out = []
                    for (g, r0, r1, p0) in raw:
                        while r0 < r1:
                            m = r1 - r0
                            if p0 == 0 or (m <= 32) or (m <= 64 and p0 == 64):
                                out.append((g, r0, r1, p0))
                                break
                            step = 32 if p0 % 64 else 64
                            step = min(step, m)
                            out.append((g, r0, r0 + step, p0))
                            r0 += step
                            p0 += step
                    return out

                for dd in range(CC):
                    for half in range(2):
                        hsl = slice(half * 384, (half + 1) * 384)
                        p_ps = psb.tile(
                            [128, 384], fp32, tag="big", name=f"pps_{dd}_{half}"
                        )
                        for (g, r0, r1, p0) in d_pieces(dd):
                            nc.tensor.matmul(
                                p_ps[p0 : p0 + (r1 - r0), :],
                                e16[g][:, r0:r1],
                                wpg_sb[g][:, hsl],
                                start=True,
                                stop=True,
                                tile_position=(0, p0) if p0 else None,
                            )
                        if dd % 2 == 0:
                            nc.scalar.copy(out=P6[dd][:, hsl], in_=p_ps)
                        else:
                            nc.vector.tensor_copy(P6[dd][:, hsl], p_ps)

                for half in range(2):
                    hsl = slice(half * 384, (half + 1) * 384)
                    for ab in range(CC):
                        m_ps = psb.tile(
                            [128, 384], fp32, tag="big", name=f"mps_{ab}_{half}"
                        )
                        for dd in range(CC):
                            nc.tensor.matmul(
                                m_ps,
                                wv_sb[dd][:, ab * 128 : (ab + 1) * 128],
                                P6[dd][:, hsl],
                                start=(dd == 0),
                                stop=(dd == CC - 1),
                            )
                        if ab % 2 == 0:
                            nc.scalar.copy(out=M_sb[ab][:, hsl], in_=m_ps)
                        else:
                            nc.vector.tensor_copy(M_sb[ab][:, hsl], m_ps)

                # ---- phase 3: y = x @ M + b (fp16 out) ----
                ydma = [nc.sync, nc.scalar, nc.gpsimd]
                for ti in range(NT):
                    r0 = ti * 128
                    y_sb = ysb_pool.tile(
                        [128, C], fp16, tag="ysb", name=f"ysb_{ti}"
                    )
                    for half in range(2):
                        hsl = slice(half * 384, (half + 1) * 384)
                        y_ps = psb.tile(
                            [128, 384], fp32, tag="big", name=f"yps_{ti}_{half}"
                        )
                        for a in range(CC):
                            nc.tensor.matmul(
                                y_ps,
                                xT6[a][:, r0 : r0 + 128],
                                M_sb[a][:, hsl],
                                start=(a == 0),
                                stop=(a == CC - 1),
                            )
                        nc.vector.tensor_add(y_sb[:, hsl], y_ps, bias_sb[:, hsl])
                    ydma[ti % 3].dma_start(out=y[r0 : r0 + 128, :], in_=y_sb)

    nc.compile()
    return nc


def _get_nc():
    if "nc" not in _CACHE:
        _CACHE["nc"] = _build_nc()
    return _CACHE["nc"]


def _host_prep(x, w_qkv, w_proj, b_proj):
    x = np.asarray(x, dtype=np.float32)
    w_qkv = np.asarray(w_qkv, dtype=np.float32)
    w_proj = np.asarray(w_proj, dtype=np.float32)
    b_proj = np.asarray(b_proj, dtype=np.float32)

    wqk = w_qkv[: 2 * C, :].copy()
    wqk[:C, :] *= np.float32(QSCALE)
    wqkT_h = np.ascontiguousarray(wqk.T).astype(np.float16)       # [768, 1536]
    wv_h = np.ascontiguousarray(w_qkv[2 * C :, :]).astype(np.float16)
    wprojT_h = np.ascontiguousarray(w_proj.T).astype(np.float16)  # [768, 768]

    id16 = np.eye(128, dtype=np.float16)
    in_maps = []
    for b_ in range(NCORES):
        xb16 = np.ascontiguousarray(x[b_]).astype(np.float16)
        in_maps.append(
            {
                "xh": xb16,
                "xTh": np.ascontiguousarray(xb16.T),
                "wqkT": wqkT_h,
                "wv": wv_h,
                "wprojT": wprojT_h,
                "bproj": b_proj,
                "id16": id16,
            }
        )
    return in_maps


def _run(in_maps, trace=False):
    from concourse.bass_utils import run_bass_kernel_spmd

    nc = _get_nc()
    res = run_bass_kernel_spmd(nc, in_maps, list(range(NCORES)), trace=trace)
    out = np.stack([res.results[i]["y"] for i in range(NCORES)], axis=0)
    return out.astype(np.float32, copy=False), res


def kernel(x, w_qkv, w_proj, b_proj):
    in_maps = _host_prep(x, w_qkv, w_proj, b_proj)
    out, _ = _run(in_maps, trace=False)
    return out


def run_profiled(x, w_qkv, w_proj, b_proj):
    """Returns (out, BassKernelResults) with NTFF profiling enabled."""
    in_maps = _host_prep(x, w_qkv, w_proj, b_proj)
    return _run(in_maps, trace=True)
